# revision 1
# baseline (speedup 1.0000x reference)
"""Batched forward-kinematics (DiffKin) Bass kernel for 8 TRN2 NeuronCores.

Problem (hardcoded): B=65536 configurations, 32-frame kinematic tree, 29 DOF,
PARENTS = [-1, 0..28, 10, 10]. Output (B, 32, 4, 4) fp32 poses.

Every structural input (origins, axes, mimic maps, joint types, ctrl indices)
is read on the host at call time and folded into per-frame constants:
    local_f = M0 + sin(phi)*M1 + cos(phi)*M2   (revolute; phi affine in one
                                                joint_angles column)
    local_f = M0 + x*M1                        (prismatic)
    local_f = M0                               (fixed / constant angle)
The reference's eps-guarded Rodrigues reduces to this within ~1e-9 abs.
Constant chain prefixes are folded on the host too.

Per core (batch slice 8192 = 128 partitions x Q=64):
 - DVE: angle affines, sin/cos range reduction (fp32 magic-number rounding),
   and the sequential batched 4x4 compose chain via 0-stride broadcast APs.
 - ACT: Sin activations (cos = sin(x + pi/2), range-wrapped).
 - GPSIMD: local-matrix materialization (scalar-column x const-row outers).
 - sync/HWDGE: DMA. Output is frame-major (32, 8192, 16) per core so each
   DMA moves 4KB-contiguous runs; the host transposes once at the end.

Raw-Bass Block style with manual tick-counter semaphores (Tile's multi-wait
instructions don't encode on this walrus). Adjacent same-engine RAW hazards
(8-deep DVE pipe, empirically confirmed) are broken with drain().
"""
import numpy as np
from contextlib import ExitStack

import concourse.bass as bass
import concourse.mybir as mybir
from concourse.bass_utils import run_bass_kernel_spmd

FP = mybir.dt.float32
Alu = mybir.AluOpType
ActF = mybir.ActivationFunctionType

B = 65536
NFRAMES = 32
DOF = 29
NCORES = 8
BC = B // NCORES
P = 128
Q = BC // P
PARENTS = [-1] + list(range(29)) + [10, 10]
ORDER = list(range(11)) + [30, 31] + list(range(11, 30))

MAGIC = float(1.5 * 2**23)
INV2PI = float(1.0 / (2 * np.pi))
TWOPI = float(2 * np.pi)
HALFPI = float(np.pi / 2)

NPOSE = 6
NLOC = 6
GPS_DRAIN = True   # drain gpsimd between dependent ops (safe default)

_cache = {}


def _skew(a):
    return np.array([[0, -a[2], a[1]], [a[2], 0, -a[0]], [-a[1], a[0], 0]],
                    dtype=np.float64)


def _plan_host(all_axes, all_origins, mimic_multipliers, mimic_offsets,
               ctrlable_indices, mimic_src_indices, mimic_dst_indices,
               joint_types):
    axes = np.asarray(all_axes, np.float64)
    origins = np.asarray(all_origins, np.float64)
    mm = np.asarray(mimic_multipliers, np.float64)
    mo = np.asarray(mimic_offsets, np.float64)
    ctrl = np.asarray(ctrlable_indices, np.int64)
    msrc = np.asarray(mimic_src_indices, np.int64)
    mdst = np.asarray(mimic_dst_indices, np.int64)
    types = np.asarray(joint_types, np.int64)

    amap = {f: (None, 0.0, 0.0) for f in range(NFRAMES)}
    for d, f in enumerate(ctrl):
        amap[int(f)] = (d, 1.0, 0.0)
    ctrl_only = dict(amap)
    for j in range(len(mdst)):
        s, dcol = int(msrc[j]), int(mdst[j])
        scol, ssc, soff = ctrl_only[s]
        if scol is None:
            amap[dcol] = (None, 0.0, float(mm[j] * soff + mo[j]))
        else:
            amap[dcol] = (scol, float(mm[j] * ssc), float(mm[j] * soff + mo[j]))

    affine = bool(np.all(origins[:, 3, :] == np.array([0, 0, 0, 1.0])))
    I4 = np.eye(4)

    def local_decomp(f):
        O = origins[f]
        col, sc, off = amap[f]
        t = int(types[f])
        a = axes[f]
        n2 = float(a @ a)
        if t == 1 and n2 > 1e-24:
            n = np.sqrt(n2)
            S = np.zeros((4, 4)); S[:3, :3] = _skew(a)
            S2 = S @ S
            M1 = O @ S / n
            M2n = O @ S2 / n2
            if col is None:
                phi = n * off
                return ('const', O + np.sin(phi) * M1 + (1 - np.cos(phi)) * M2n)
            return ('rev', O + M2n, M1, -M2n, n * sc, n * off)
        if t == 2 and n2 > 0:
            A = np.zeros((4, 4)); A[:3, 3] = a
            M1 = O @ A
            if col is None:
                return ('const', O + off * M1)
            return ('prism', O, M1, sc, off)
        return ('const', O)

    nodes = []
    pose_const = {}
    for f in ORDER:
        p = PARENTS[f]
        Cp = I4 if p < 0 else pose_const.get(p)
        dec = local_decomp(f)
        if dec[0] == 'const':
            L = dec[1]
            if Cp is not None:
                M = Cp @ L
                pose_const[f] = M
                nodes.append(dict(kind='cpose', f=f, M=M))
            else:
                nodes.append(dict(kind='ccomp', f=f, parent=p, L=L))
        elif dec[0] == 'rev':
            _, M0, M1, M2, sc, off = dec
            kw = dict(f=f, col=amap[f][0], sc=sc, off=off, trig=True)
            if Cp is not None:
                nodes.append(dict(kind='mat', M0=Cp @ M0, M1=Cp @ M1, M2=Cp @ M2, **kw))
            else:
                nodes.append(dict(kind='comp', parent=p, M0=M0, M1=M1, M2=M2, **kw))
        else:
            _, M0, M1, sc, off = dec
            kw = dict(f=f, col=amap[f][0], sc=sc, off=off, trig=False)
            if Cp is not None:
                nodes.append(dict(kind='mat', M0=Cp @ M0, M1=Cp @ M1, M2=None, **kw))
            else:
                nodes.append(dict(kind='comp', parent=p, M0=M0, M1=M1, M2=None, **kw))
    return nodes, affine


def _build_program(nodes, affine):
    EL = 12 if affine else 16
    EL3 = 3 if affine else 4

    def mrow(M):
        M = np.asarray(M, np.float64)
        return M[:3, :].reshape(12) if affine else M.reshape(16)

    consts = []
    def cpush(vec):
        off = len(consts)
        consts.extend(float(x) for x in vec)
        return off

    trig_nodes = [nd for nd in nodes if nd['kind'] in ('mat', 'comp') and nd['trig']]
    lin_nodes = [nd for nd in nodes if nd['kind'] in ('mat', 'comp') and not nd['trig']]
    for i, nd in enumerate(trig_nodes):
        nd['ti'] = i
    for i, nd in enumerate(lin_nodes):
        nd['li'] = i
    NT = len(trig_nodes)
    NLIN = len(lin_nodes)

    for nd in nodes:
        if nd['kind'] == 'cpose':
            nd['c_M'] = cpush(np.asarray(nd['M'], np.float64).reshape(16))
        elif nd['kind'] == 'ccomp':
            nd['c_L'] = cpush(mrow(nd['L']))
        else:
            nd['c_M0'] = cpush(mrow(nd['M0']))
            nd['c_M1'] = cpush(mrow(nd['M1']))
            if nd['M2'] is not None:
                nd['c_M2'] = cpush(mrow(nd['M2']))
    NC = len(consts)
    consts_vec = np.asarray(consts, np.float32)

    frame_to_node = {nd['f']: nd for nd in nodes}
    for oi, nd in enumerate(nodes):
        nd['oi'] = oi
        nd['buf'] = oi % NPOSE

    # ---- plan ticks ----
    n_memset = 2 * NPOSE if affine else 0
    gt = n_memset
    var_idx = 0
    for nd in nodes:
        if nd['kind'] == 'cpose':
            gt += 1
            nd['g_done'] = gt
        elif nd['kind'] == 'mat':
            gt += (4 if nd['trig'] else 2) + 1  # +1 row shim? no -- keep exact below
            gt -= 1
            nd['g_done'] = gt
        elif nd['kind'] == 'comp':
            nd['lbuf'] = var_idx % NLOC
            nd['lseq'] = var_idx
            var_idx += 1
            gt += (4 if nd['trig'] else 2)
            nd['g_done'] = gt
    n_angle_dve = NT + NLIN + (7 if NT else 0)
    vt = n_angle_dve
    for nd in nodes:
        if nd['kind'] in ('ccomp', 'comp'):
            vt += 6 if affine else 7
            nd['v_done'] = vt
    lseq_to_node = {nd.get('lseq'): nd for nd in nodes if nd.get('lseq') is not None}
    comp_nodes = [nd for nd in nodes if nd['kind'] in ('ccomp', 'comp')]

    def prior_uses(nd):
        return sum(1 for n2 in nodes[:nd['oi']] if n2['buf'] == nd['buf'])

    nc = bass.Bass()
    ja_in = nc.declare_dram_parameter("ja", [BC, DOF], FP, isOutput=False)
    co_in = nc.declare_dram_parameter("co", [P, max(NC, 4)], FP, isOutput=False)
    out_d = nc.declare_dram_parameter("out", [NFRAMES, BC, 16], FP, isOutput=True)

    with ExitStack() as st:
        def sb(name, shape):
            return st.enter_context(nc.sbuf_tensor(name, shape, FP))

        ja = sb("ja_t", [P, Q, DOF])
        co = sb("co_t", [P, max(NC, 4)])
        NTx = max(NT, 1)
        phi = sb("phi_t", [P, Q, NTx])
        w1 = sb("w1_t", [P, Q, NTx])
        w2 = sb("w2_t", [P, Q, NTx])
        w3 = sb("w3_t", [P, Q, NTx])
        argS = sb("argS_t", [P, Q, NTx])
        argC = sb("argC_t", [P, Q, NTx])
        sinv = sb("sin_t", [P, Q, NTx])
        cosv = sb("cos_t", [P, Q, NTx])
        xlin = sb("xlin_t", [P, Q, max(NLIN, 1)])
        poses = [sb(f"pose{i}", [P, Q, 16]) for i in range(NPOSE)]
        locs = [sb(f"loc{i}", [P, Q, EL]) for i in range(NLOC)]
        tA = sb("tA_t", [P, Q, EL])
        tB = sb("tB_t", [P, Q, EL])
        gsA = sb("gsA_t", [P, Q, EL])

        in_sem = st.enter_context(nc.semaphore(name="in_sem"))
        v_sem = st.enter_context(nc.semaphore(name="v_sem"))
        g_sem = st.enter_context(nc.semaphore(name="g_sem"))
        a_sem = st.enter_context(nc.semaphore(name="a_sem"))
        pd_sems = [st.enter_context(nc.semaphore(name=f"pd{i}")) for i in range(NPOSE)]
        block = st.enter_context(nc.Block())

        def r4(t):
            return t[:].rearrange("p q (i j) -> p q i j", j=4)

        def T(ap):
            # iterate with q innermost: (P, Q, i, j) -> (P, i, j, Q)
            return ap.transpose([0, 2, 3, 1])

        def T3(ap):
            # (P, Q, e) -> (P, e, Q)
            return ap.transpose([0, 2, 1])

        # ---------------- sync: DMA ----------------
        @block.sync
        def _(sync):
            sync.dma_start(out=ja[:], in_=ja_in[:].rearrange("(p q) d -> p q d", p=P)
                           ).then_inc(in_sem, 16)
            sync.dma_start(out=co[:], in_=co_in[:]).then_inc(in_sem, 16)
            uses = [0] * NPOSE
            for nd in nodes:
                b = nd['buf']
                if nd['kind'] in ('cpose', 'mat'):
                    sync.wait_ge(g_sem, nd['g_done'])
                else:
                    sync.wait_ge(v_sem, nd['v_done'])
                    if affine and uses[b] == 0:
                        sync.wait_ge(g_sem, n_memset)
                sync.dma_start(
                    out=out_d[nd['f']].rearrange("(p q) e -> p q e", p=P),
                    in_=poses[b][:],
                ).then_inc(pd_sems[b], 16)
                uses[b] += 1
            for i in range(NPOSE):
                if uses[i]:
                    sync.wait_ge(pd_sems[i], 16 * uses[i])

        # ---------------- vector: angles + composes ----------------
        @block.vector
        def _(vector):
            # Self-sem ordering for same-engine RAW/WAW/WAR: then_inc fires at
            # instruction retirement, so a wait_ge on the own tick both flushes
            # the 8-deep DVE pipe and satisfies CoreSim's race model.
            state = dict(v=0, waited=0, wtick={}, rtick={})

            def op(fn, writes, reads):
                need = 0
                for k in frozenset(writes) | frozenset(reads):
                    need = max(need, state['wtick'].get(k, 0))
                for k in frozenset(writes):
                    need = max(need, state['rtick'].get(k, 0))
                if need > state['waited']:
                    vector.wait_ge(v_sem, need)
                    state['waited'] = need
                inst = fn()
                inst.then_inc(v_sem, 1)
                state['v'] += 1
                for k in frozenset(writes):
                    state['wtick'][k] = state['v']
                for k in frozenset(reads):
                    state['rtick'][k] = max(state['rtick'].get(k, 0), state['v'])

            vector.wait_ge(in_sem, 32)
            for nd in trig_nodes:
                i = nd['ti']
                op(lambda nd=nd, i=i: nc.vector.tensor_scalar(
                    out=phi[:, :, i], in0=ja[:, :, nd['col']],
                    scalar1=float(nd['sc']), scalar2=float(nd['off']),
                    op0=Alu.mult, op1=Alu.add), {'phi'}, {'ja'})
            for nd in lin_nodes:
                i = nd['li']
                op(lambda nd=nd, i=i: nc.vector.tensor_scalar(
                    out=xlin[:, :, i], in0=ja[:, :, nd['col']],
                    scalar1=float(nd['sc']), scalar2=float(nd['off']),
                    op0=Alu.mult, op1=Alu.add), {'xlin'}, {'ja'})
            if NT:
                PHI = phi[:, :, 0:NT]
                op(lambda: nc.vector.tensor_scalar(out=w1[:, :, 0:NT], in0=PHI,
                   scalar1=INV2PI, scalar2=MAGIC, op0=Alu.mult, op1=Alu.add),
                   {'w1'}, {'phi'})
                op(lambda: nc.vector.tensor_scalar_add(out=w2[:, :, 0:NT],
                   in0=w1[:, :, 0:NT], scalar1=-MAGIC), {'w2'}, {'w1'})
                op(lambda: nc.vector.scalar_tensor_tensor(out=argS[:, :, 0:NT],
                   in0=w2[:, :, 0:NT], scalar=-TWOPI, in1=PHI,
                   op0=Alu.mult, op1=Alu.add), {'argS'}, {'w2', 'phi'})
                op(lambda: nc.vector.tensor_scalar_add(out=w1[:, :, 0:NT],
                   in0=PHI, scalar1=HALFPI), {'w1'}, {'phi'})
                op(lambda: nc.vector.tensor_scalar(out=w2[:, :, 0:NT],
                   in0=w1[:, :, 0:NT], scalar1=INV2PI, scalar2=MAGIC,
                   op0=Alu.mult, op1=Alu.add), {'w2'}, {'w1'})
                op(lambda: nc.vector.tensor_scalar_add(out=w3[:, :, 0:NT],
                   in0=w2[:, :, 0:NT], scalar1=-MAGIC), {'w3'}, {'w2'})
                op(lambda: nc.vector.scalar_tensor_tensor(out=argC[:, :, 0:NT],
                   in0=w3[:, :, 0:NT], scalar=-TWOPI, in1=w1[:, :, 0:NT],
                   op0=Alu.mult, op1=Alu.add), {'argC'}, {'w3', 'w1'})
            assert state['v'] == n_angle_dve, (state['v'], n_angle_dve)

            for nd in comp_nodes:
                b = nd['buf']
                pnd = frame_to_node[nd['parent']]
                pb = pnd['buf']
                pu = prior_uses(nd)
                if pu:
                    vector.wait_ge(pd_sems[b], 16 * pu)
                if pnd['kind'] in ('cpose', 'mat'):
                    vector.wait_ge(g_sem, pnd['g_done'])
                if nd['kind'] == 'comp':
                    vector.wait_ge(g_sem, nd['g_done'])
                    Lv = r4(locs[nd['lbuf']])
                    def lrow(k, Lv=Lv):
                        return Lv[:, :, k, :].unsqueeze(2).broadcast_to([P, Q, EL3, 4])
                    lkey = ('loc', nd['lbuf'])
                else:
                    o = nd['c_L']
                    def lrow(k, o=o):
                        return co[:, o + 4 * k: o + 4 * k + 4].unsqueeze(1).unsqueeze(2) \
                            .broadcast_to([P, Q, EL3, 4])
                    lkey = 'co'
                Cv = r4(poses[b])
                Pv = r4(poses[pb])
                Cw = Cv[:, :, 0:EL3, :]
                def prow(k, Pv=Pv):
                    return Pv[:, :, 0:EL3, k].unsqueeze(3).broadcast_to([P, Q, EL3, 4])
                bk = ('pose', b)
                pk = ('pose', pb)
                tAv = r4(tA)
                tBv = r4(tB)
                op(lambda: nc.vector.tensor_tensor(out=T(Cw), in0=T(prow(0)), in1=T(lrow(0)),
                   op=Alu.mult), {bk}, {pk, lkey})
                op(lambda: nc.vector.tensor_tensor(out=T(tAv), in0=T(prow(1)), in1=T(lrow(1)),
                   op=Alu.mult), {'tA'}, {pk, lkey})
                op(lambda: nc.vector.tensor_tensor(out=T(tBv), in0=T(prow(2)), in1=T(lrow(2)),
                   op=Alu.mult), {'tB'}, {pk, lkey})
                if affine:
                    op(lambda: nc.vector.tensor_tensor(out=T3(Cv[:, :, 0:3, 3]),
                       in0=T3(Cv[:, :, 0:3, 3]), in1=T3(Pv[:, :, 0:3, 3]), op=Alu.add),
                       {bk}, {bk, pk})
                    op(lambda: nc.vector.tensor_tensor(out=T(Cw), in0=T(Cw), in1=T(tAv),
                       op=Alu.add), {bk}, {bk, 'tA'})
                    op(lambda: nc.vector.tensor_tensor(out=T(Cw), in0=T(Cw), in1=T(tBv),
                       op=Alu.add), {bk}, {bk, 'tB'})
                else:
                    op(lambda: nc.vector.tensor_tensor(out=T(Cw), in0=T(Cw), in1=T(tAv),
                       op=Alu.add), {bk}, {bk, 'tA'})
                    op(lambda: nc.vector.tensor_tensor(out=T(tAv), in0=T(prow(3)), in1=T(lrow(3)),
                       op=Alu.mult), {'tA'}, {pk, lkey})
                    op(lambda: nc.vector.tensor_tensor(out=T(Cw), in0=T(Cw), in1=T(tBv),
                       op=Alu.add), {bk}, {bk, 'tB'})
                    op(lambda: nc.vector.tensor_tensor(out=T(Cw), in0=T(Cw), in1=T(tAv),
                       op=Alu.add), {bk}, {bk, 'tA'})
                assert state['v'] == nd['v_done'], (nd['f'], state['v'], nd['v_done'])

        # ---------------- gpsimd: locals + const poses ----------------
        @block.gpsimd
        def _(gpsimd):
            # Pool ops can overlap across the 8 DSP cores: same-engine RAW/WAW
            # needs explicit self-sem ordering (CoreSim race model agrees).
            gstate = dict(g=0, waited=0, wtick={}, rtick={})

            def gop(fn, writes, reads):
                need = 0
                for k in frozenset(writes) | frozenset(reads):
                    need = max(need, gstate['wtick'].get(k, 0))
                for k in frozenset(writes):
                    need = max(need, gstate['rtick'].get(k, 0))
                if need > gstate['waited']:
                    gpsimd.wait_ge(g_sem, need)
                    gstate['waited'] = need
                inst = fn()
                inst.then_inc(g_sem, 1)
                gstate['g'] += 1
                for k in frozenset(writes):
                    gstate['wtick'][k] = gstate['g']
                for k in frozenset(reads):
                    gstate['rtick'][k] = max(gstate['rtick'].get(k, 0), gstate['g'])

            gpsimd.wait_ge(in_sem, 32)
            if affine:
                for bi, pt in enumerate(poses):
                    pv = r4(pt)
                    gop(lambda pv=pv: gpsimd.memset(T3(pv[:, :, 3, 0:3]), 0.0),
                        {('pose', bi)}, set())
                    gop(lambda pv=pv: gpsimd.memset(pv[:, :, 3, 3], 1.0),
                        {('pose', bi)}, set())
            assert gstate['g'] == n_memset
            waited_act = False
            waited_lin = False
            for nd in nodes:
                if nd['kind'] == 'ccomp':
                    continue
                b = nd['buf']
                if nd['kind'] == 'cpose':
                    pu = prior_uses(nd)
                    if pu:
                        gpsimd.wait_ge(pd_sems[b], 16 * pu)
                    o = nd['c_M']
                    gop(lambda o=o, b=b: nc.gpsimd.tensor_copy(
                        out=T3(poses[b][:]),
                        in_=T3(co[:, o:o + 16].unsqueeze(1).broadcast_to([P, Q, 16]))),
                        {('pose', b)}, {'co'})
                    assert gstate['g'] == nd['g_done']
                    continue
                # mat / comp
                if nd['trig'] and not waited_act:
                    gpsimd.wait_ge(a_sem, 2)
                    waited_act = True
                if not nd['trig'] and not waited_lin:
                    gpsimd.wait_ge(v_sem, NT + NLIN)
                    waited_lin = True
                if nd['kind'] == 'mat':
                    pu = prior_uses(nd)
                    if pu:
                        gpsimd.wait_ge(pd_sems[b], 16 * pu)
                    out_t = r4(poses[b])[:, :, 0:EL3, :]
                    okey = ('pose', b)
                else:
                    ls = nd['lseq']
                    if ls >= NLOC:
                        gpsimd.wait_ge(v_sem, lseq_to_node[ls - NLOC]['v_done'])
                    out_t = r4(locs[nd['lbuf']])
                    okey = ('loc', nd['lbuf'])

                def crow(o):
                    return co[:, o:o + EL].unsqueeze(1).broadcast_to([P, Q, EL]) \
                        .rearrange("p q (i j) -> p q i j", j=4)

                if nd['trig']:
                    i = nd['ti']
                    sv = sinv[:, :, i].unsqueeze(2).broadcast_to([P, Q, EL]) \
                        .rearrange("p q (i j) -> p q i j", j=4)
                    cv = cosv[:, :, i].unsqueeze(2).broadcast_to([P, Q, EL]) \
                        .rearrange("p q (i j) -> p q i j", j=4)
                    gAv = r4(gsA)
                    gop(lambda nd=nd, sv=sv, out_t=out_t: nc.gpsimd.tensor_tensor(
                        out=T(out_t), in0=T(sv), in1=T(crow(nd['c_M1'])), op=Alu.mult),
                        {okey}, {'sin', 'co'})
                    gop(lambda nd=nd, cv=cv, gAv=gAv: nc.gpsimd.tensor_tensor(
                        out=T(gAv), in0=T(cv), in1=T(crow(nd['c_M2'])), op=Alu.mult),
                        {'gsA'}, {'cos', 'co'})
                    gop(lambda out_t=out_t, gAv=gAv: nc.gpsimd.tensor_tensor(
                        out=T(out_t), in0=T(out_t), in1=T(gAv), op=Alu.add),
                        {okey}, {okey, 'gsA'})
                    gop(lambda nd=nd, out_t=out_t: nc.gpsimd.tensor_tensor(
                        out=T(out_t), in0=T(out_t), in1=T(crow(nd['c_M0'])), op=Alu.add),
                        {okey}, {okey, 'co'})
                else:
                    i = nd['li']
                    xv = xlin[:, :, i].unsqueeze(2).broadcast_to([P, Q, EL]) \
                        .rearrange("p q (i j) -> p q i j", j=4)
                    gop(lambda nd=nd, xv=xv, out_t=out_t: nc.gpsimd.tensor_tensor(
                        out=T(out_t), in0=T(xv), in1=T(crow(nd['c_M1'])), op=Alu.mult),
                        {okey}, {'xlin', 'co'})
                    gop(lambda nd=nd, out_t=out_t: nc.gpsimd.tensor_tensor(
                        out=T(out_t), in0=T(out_t), in1=T(crow(nd['c_M0'])), op=Alu.add),
                        {okey}, {okey, 'co'})
                assert gstate['g'] == nd['g_done'], (nd['f'], gstate['g'], nd['g_done'])

        # ---------------- scalar: sin/cos ----------------
        @block.scalar
        def _(scalar):
            if NT:
                scalar.wait_ge(v_sem, NT + NLIN + 3)
                nc.scalar.activation(out=sinv[:, :, 0:NT], in_=argS[:, :, 0:NT],
                                     func=ActF.Sin, bias=0.0, scale=1.0
                                     ).then_inc(a_sem, 1)
                scalar.wait_ge(v_sem, NT + NLIN + 7)
                nc.scalar.activation(out=cosv[:, :, 0:NT], in_=argC[:, :, 0:NT],
                                     func=ActF.Sin, bias=0.0, scale=1.0
                                     ).then_inc(a_sem, 1)

    return nc, consts_vec, NC


def _get_program(inputs):
    key_parts = []
    for name in ("all_axes", "all_origins", "mimic_multipliers", "mimic_offsets",
                 "ctrlable_indices", "mimic_src_indices", "mimic_dst_indices",
                 "joint_types"):
        key_parts.append(np.asarray(inputs[name]).tobytes())
    key = hash(tuple(key_parts))
    if key not in _cache:
        nodes, affine = _plan_host(
            inputs["all_axes"], inputs["all_origins"], inputs["mimic_multipliers"],
            inputs["mimic_offsets"], inputs["ctrlable_indices"],
            inputs["mimic_src_indices"], inputs["mimic_dst_indices"],
            inputs["joint_types"])
        nc, consts_vec, NC = _build_program(nodes, affine)
        co_arr = np.zeros((P, max(NC, 4)), np.float32)
        co_arr[:, :NC] = consts_vec[None, :]
        _cache[key] = (nc, co_arr)
    return _cache[key]


def kernel(**inputs):
    ja = np.ascontiguousarray(np.asarray(inputs["joint_angles"], np.float32))
    assert ja.shape == (B, DOF)
    nc, co_arr = _get_program(inputs)
    in_maps = [{"ja": np.ascontiguousarray(ja[c * BC:(c + 1) * BC]), "co": co_arr}
               for c in range(NCORES)]
    res = run_bass_kernel_spmd(nc, in_maps, list(range(NCORES))).results
    full = np.stack([r["out"] for r in res])          # (8, 32, 8192, 16)
    out = full.transpose(0, 2, 1, 3).reshape(B, NFRAMES, 4, 4)
    return np.ascontiguousarray(out)

